# revision 6
# baseline (speedup 1.0000x reference)
"""ActionCoherenceLoss kernel for 8 Trainium2 NeuronCores.

reference:
    norm = ||x||_2 along D; h = x / max(norm, eps)
    diag_sim[b, l] = <h[b,l], h[b,l+1]>          (l = 0..L-2)
    out = 1 - mean(diag_sim)                      (f32 scalar)

Strategy:
  - Data-parallel over batch: core b handles x[b] ([L=4096, D=2048]).
  - Host: transpose to x^T [D, L], pad one zero row -> [D, L+1], cast bf16.
  - Device: for each 128-row block i, compute the near-diagonal Gram block
        G_i = X_blk^T @ X_blk'  in PSUM  ([128, 129], fp32 accum over 16
        feature chunks of 128).  diag(G_i)[p] = s_{128i+p} = ||x_l||^2,
        superdiag(G_i)[p] = c_{128i+p} = <x_l, x_{l+1}>.
    Extract both diagonals with a fused masked multiply+reduce on VectorE.
  - Host: combine s, c from all 8 cores in float64:
        diag_sim_l = c_l / (max(sqrt(s_l),eps) * max(sqrt(s_{l+1}),eps))
"""

import numpy as np
import ml_dtypes

B, L, D = 8, 4096, 2048
P = 128
NCHUNK = D // P                # 16 feature chunks
SLABS = 8                      # row-range pipeline granularity for input DMA
SLAB_ROWS = L // SLABS         # 512
SLAB_COLS = SLAB_ROWS + 1      # 513 (one row overlap / zero pad at the end)
NBLK = L // P                  # 32 Gram blocks per core
BLK_PER_SLAB = NBLK // SLABS   # 4
EPS = 1e-12
IN_DT = "float8e4"             # input dtype on device: float8e4 | bfloat16

_cache = {}


def _build():
    import concourse.bass as bass
    import concourse.bacc as bacc
    import concourse.tile as tile
    from concourse import mybir

    nc = bacc.Bacc(
        "TRN2", target_bir_lowering=False, debug=False, num_devices=B
    )
    f32 = mybir.dt.float32
    in_dt = getattr(mybir.dt, IN_DT)

    xt_d = nc.dram_tensor(
        "xt", [NCHUNK, SLABS, P, SLAB_COLS], in_dt, kind="ExternalInput"
    ).ap()
    mk_d = nc.dram_tensor("mk", [P, 2 * (P + 1)], f32, kind="ExternalInput").ap()
    sc_d = nc.dram_tensor("sc", [P, 2 * NBLK], f32, kind="ExternalOutput").ap()

    with tile.TileContext(nc) as tc:
        with (
            tc.tile_pool(name="xin", bufs=1) as xin,
            tc.tile_pool(name="cst", bufs=1) as cst,
            tc.tile_pool(name="scr", bufs=4) as scr,
            tc.tile_pool(name="outp", bufs=1) as outp,
            tc.tile_pool(name="psum", bufs=8, space=bass.MemorySpace.PSUM) as psum,
        ):
            mk = cst.tile([P, 2 * (P + 1)], f32, name="mk_sb")
            nc.sync.dma_start(out=mk, in_=mk_d)
            sc = outp.tile([P, 2 * NBLK], f32, name="sc_sb")

            # Input tiles, DMA'd slab-major so early row blocks are ready
            # while later slabs stream in.
            xt = {}
            for j in range(SLABS):
                for k in range(NCHUNK):
                    t = xin.tile(
                        [P, SLAB_COLS], in_dt, tag=f"xt_{k}_{j}", name=f"xt_{k}_{j}"
                    )
                    nc.sync.dma_start(out=t, in_=xt_d[k, j])
                    xt[(k, j)] = t

            for i in range(NBLK):
                j, m = divmod(i, BLK_PER_SLAB)
                m0 = m * P
                pb = psum.tile([P, P + 1], f32, tag="gram", name=f"gram_{i}")
                for k in range(NCHUNK):
                    t = xt[(k, j)]
                    nc.tensor.matmul(
                        pb,
                        t[:, m0 : m0 + P],          # lhsT: stationary
                        t[:, m0 : m0 + P + 1],      # rhs: moving
                        start=(k == 0),
                        stop=(k == NCHUNK - 1),
                    )
                for h in range(2):  # 0 -> diag (s), 1 -> superdiag (c)
                    tmp = scr.tile(
                        [P, P + 1], f32, tag="scr", name=f"scr_{i}_{h}"
                    )
                    col = h * NBLK + i
                    nc.vector.tensor_mul(
                        tmp, pb, mk[:, h * (P + 1) : (h + 1) * (P + 1)]
                    )
                    nc.vector.reduce_sum(
                        sc[:, col : col + 1], tmp, axis=mybir.AxisListType.X
                    )

            nc.sync.dma_start(out=sc_d, in_=sc)
    nc.compile()
    return nc


def _make_masks():
    mk = np.zeros((P, 2 * (P + 1)), np.float32)
    r = np.arange(P)
    mk[r, r] = 1.0                  # diag mask (cols 0..128)
    mk[r, (P + 1) + r + 1] = 1.0    # superdiag mask (cols 129..257)
    return mk


def _np_in_dt():
    return {"float8e4": ml_dtypes.float8_e4m3, "bfloat16": ml_dtypes.bfloat16}[IN_DT]


def _prep_inputs(x):
    """x: [B, L, D] float32 -> list of per-core input maps."""
    np_dt = _np_in_dt()
    mk = _make_masks()
    in_maps = []
    for b in range(B):
        xt = np.zeros((D, L + 1), dtype=np_dt)
        xt[:, :L] = np.ascontiguousarray(x[b].T).astype(np_dt)
        slabs = np.empty((NCHUNK, SLABS, P, SLAB_COLS), dtype=np_dt)
        for j in range(SLABS):
            sl = xt[:, SLAB_ROWS * j : SLAB_ROWS * j + SLAB_COLS]
            slabs[:, j] = sl.reshape(NCHUNK, P, SLAB_COLS)
        in_maps.append({"xt": slabs, "mk": mk})
    return in_maps


def _combine(results):
    total = 0.0
    for b in range(B):
        sc = np.asarray(results[b]["sc"], dtype=np.float64)  # [P, 2*NBLK]
        s = sc[:, :NBLK].T.reshape(-1)  # s_l at [l % P, l // P]
        c = sc[:, NBLK:].T.reshape(-1)
        n = np.maximum(np.sqrt(s), EPS)
        diag = c[: L - 1] / (n[: L - 1] * n[1:L])
        total += diag.sum()
    coherence = total / (B * (L - 1))
    return np.array(1.0 - coherence, dtype=np.float32)


def _run(x, trace=False):
    from concourse import bass_utils

    if "nc" not in _cache:
        _cache["nc"] = _build()
    nc = _cache["nc"]
    in_maps = _prep_inputs(np.asarray(x, dtype=np.float32))
    res = bass_utils.run_bass_kernel_spmd(
        nc, in_maps, core_ids=list(range(B)), trace=trace
    )
    return _combine(res.results), res


def kernel(hidden_states):
    out, _ = _run(hidden_states, trace=False)
    return out


# revision 8
# speedup vs baseline: 1.2102x; 1.2102x over previous
"""ActionCoherenceLoss kernel for 8 Trainium2 NeuronCores.

reference:
    norm = ||x||_2 along D; h = x / max(norm, eps)
    diag_sim[b, l] = <h[b,l], h[b,l+1]>          (l = 0..L-2)
    out = 1 - mean(diag_sim)                      (f32 scalar)

Strategy:
  - Data-parallel over batch: core b handles x[b] ([L=4096, D=2048]).
  - Host: transpose to x^T [D, L], pad one zero row -> [D, L+1], cast to
    bf16, and lay out as per-(chunk, slab) contiguous regions.
  - Device: for each 128-row block i, compute the near-diagonal Gram block
        G_i = X_blk^T @ X_blk'  in PSUM  ([128, 129], fp32 accum over 16
        feature chunks of 128).  diag(G_i)[p] = s_{128i+p} = ||x_l||^2,
        superdiag(G_i)[p] = c_{128i+p} = <x_l, x_{l+1}>.
    Two blocks share one PSUM bank ([128, 258]) so the masked
    multiply+reduce extraction on VectorE amortizes per-op overhead.
  - Host: combine s, c from all 8 cores in float64:
        diag_sim_l = c_l / (max(sqrt(s_l),eps) * max(sqrt(s_{l+1}),eps))
"""

import numpy as np
import ml_dtypes

B, L, D = 8, 4096, 2048
P = 128
NCHUNK = D // P                # 16 feature chunks
NBLK = L // P                  # 32 Gram blocks per core
EPS = 1e-12
IN_DT = "bfloat16"             # input dtype on device: bfloat16 | float8e4

# Row slabs (DMA granularity). First slab is small so the PE can start
# early; each slab covers its blocks' rows plus one lookahead row.
SLAB_BLKS = [2, 6, 6, 6, 6, 6]           # blocks per slab (sum = 32)
SLAB_START = [0, 256, 1024, 1792, 2560, 3328]
SLAB_LEN = [n * P + 1 for n in SLAB_BLKS]  # 257, 769 x5
NSLAB = len(SLAB_BLKS)
TOTCOL = sum(SLAB_LEN)                   # 4102


def _block_slab(i):
    """block index -> (slab j, column offset within slab)."""
    if i < 2:
        return 0, i * P
    j = 1 + (i - 2) // 6
    return j, i * P - SLAB_START[j]


_cache = {}


def _build():
    import concourse.bass as bass
    import concourse.bacc as bacc
    import concourse.tile as tile
    from concourse import mybir

    nc = bacc.Bacc(
        "TRN2", target_bir_lowering=False, debug=False, num_devices=B
    )
    f32 = mybir.dt.float32
    in_dt = getattr(mybir.dt, IN_DT)
    W = P + 1  # 129

    xt_d = nc.dram_tensor(
        "xt", [NCHUNK * TOTCOL * P], in_dt, kind="ExternalInput"
    ).ap()
    mk_d = nc.dram_tensor("mk", [P, 2 * 2 * W], f32, kind="ExternalInput").ap()
    sc_d = nc.dram_tensor("sc", [P, 2 * NBLK], f32, kind="ExternalOutput").ap()

    with tile.TileContext(nc) as tc:
        with (
            tc.tile_pool(name="xin", bufs=1) as xin,
            tc.tile_pool(name="cst", bufs=1) as cst,
            tc.tile_pool(name="scr", bufs=4) as scr,
            tc.tile_pool(name="outp", bufs=1) as outp,
            tc.tile_pool(name="psum", bufs=8, space=bass.MemorySpace.PSUM) as psum,
        ):
            mk = cst.tile([P, 2 * 2 * W], f32, name="mk_sb")
            nc.sync.dma_start(out=mk, in_=mk_d)
            sc = outp.tile([P, 2 * NBLK], f32, name="sc_sb")

            # Input tiles, DMA'd slab-major so early row blocks are ready
            # while later slabs stream in.  Each (chunk, slab) region is a
            # fully contiguous DRAM run.
            xt = {}
            for j in range(NSLAB):
                for k in range(NCHUNK):
                    t = xin.tile(
                        [P, SLAB_LEN[j]], in_dt,
                        tag=f"xt_{k}_{j}", name=f"xt_{k}_{j}",
                    )
                    off = (k * TOTCOL + sum(SLAB_LEN[:j])) * P
                    src = xt_d[off : off + P * SLAB_LEN[j]].rearrange(
                        "(p c) -> p c", p=P
                    )
                    nc.sync.dma_start(out=t, in_=src)
                    xt[(k, j)] = t

            for tpair in range(NBLK // 2):
                pb = psum.tile([P, 2 * W], f32, tag="gram", name=f"gram_{tpair}")
                for u in range(2):
                    i = 2 * tpair + u
                    j, m0 = _block_slab(i)
                    for k in range(NCHUNK):
                        t = xt[(k, j)]
                        nc.tensor.matmul(
                            pb[:, u * W : (u + 1) * W],
                            t[:, m0 : m0 + P],        # lhsT: stationary
                            t[:, m0 : m0 + W],        # rhs: moving
                            start=(k == 0),
                            stop=(k == NCHUNK - 1),
                        )
                for h in range(2):  # 0 -> diag (s), 1 -> superdiag (c)
                    tmp = scr.tile(
                        [P, 2 * W], f32, tag="scr", name=f"scr_{tpair}_{h}"
                    )
                    col = h * NBLK + 2 * tpair
                    nc.vector.tensor_mul(
                        tmp, pb, mk[:, 2 * h * W : 2 * (h + 1) * W]
                    )
                    nc.vector.reduce_sum(
                        sc[:, col : col + 2],
                        tmp[:].rearrange("p (b c) -> p b c", b=2),
                        axis=mybir.AxisListType.X,
                    )

            nc.sync.dma_start(out=sc_d, in_=sc)
    nc.compile()
    return nc


def _make_masks():
    W = P + 1
    mk = np.zeros((P, 2, 2, W), np.float32)
    r = np.arange(P)
    mk[r, 0, :, r] = 1.0      # diag mask, replicated for both blocks
    mk[r, 1, :, r + 1] = 1.0  # superdiag mask
    return mk.reshape(P, 2 * 2 * W)


def _np_in_dt():
    return {"float8e4": ml_dtypes.float8_e4m3, "bfloat16": ml_dtypes.bfloat16}[IN_DT]


def _prep_inputs(x):
    """x: [B, L, D] float32 -> list of per-core input maps."""
    np_dt = _np_in_dt()
    mk = _make_masks()
    in_maps = []
    for b in range(B):
        xt = np.zeros((D, L + 1), dtype=np_dt)
        xt[:, :L] = np.ascontiguousarray(x[b].T).astype(np_dt)
        # Each (chunk k, slab j) DRAM region is xt[128k:128k+128,
        # S_j:S_j+len_j] stored [P, len] row-major.
        arr = np.empty((NCHUNK, TOTCOL * P), dtype=np_dt)
        for j in range(NSLAB):
            c0 = sum(SLAB_LEN[:j])
            ln = SLAB_LEN[j]
            sl = xt[:, SLAB_START[j] : SLAB_START[j] + ln]  # [D, ln]
            arr[:, c0 * P : (c0 + ln) * P] = sl.reshape(NCHUNK, P * ln)
        in_maps.append({"xt": arr.reshape(-1), "mk": mk})
    return in_maps


def _combine(results):
    total = 0.0
    for b in range(B):
        sc = np.asarray(results[b]["sc"], dtype=np.float64)  # [P, 2*NBLK]
        s = sc[:, :NBLK].T.reshape(-1)  # s_l at [l % P, l // P]
        c = sc[:, NBLK:].T.reshape(-1)
        n = np.maximum(np.sqrt(s), EPS)
        diag = c[: L - 1] / (n[: L - 1] * n[1:L])
        total += diag.sum()
    coherence = total / (B * (L - 1))
    return np.array(1.0 - coherence, dtype=np.float32)


def _run(x, trace=False):
    from concourse import bass_utils

    if "nc" not in _cache:
        _cache["nc"] = _build()
    nc = _cache["nc"]
    in_maps = _prep_inputs(np.asarray(x, dtype=np.float32))
    res = bass_utils.run_bass_kernel_spmd(
        nc, in_maps, core_ids=list(range(B)), trace=trace
    )
    return _combine(res.results), res


def kernel(hidden_states):
    out, _ = _run(hidden_states, trace=False)
    return out


# revision 9
# speedup vs baseline: 1.5191x; 1.2553x over previous
"""ActionCoherenceLoss kernel for 8 Trainium2 NeuronCores.

reference:
    norm = ||x||_2 along D; h = x / max(norm, eps)
    diag_sim[b, l] = <h[b,l], h[b,l+1]>          (l = 0..L-2)
    out = 1 - mean(diag_sim)                      (f32 scalar)

Strategy:
  - Data-parallel over batch: core b handles x[b] ([L=4096, D=2048]).
  - Host: transpose to x^T [D, L], pad one zero row -> [D, L+1], cast to
    bf16, and pack so each DMA is one contiguous ~1 MiB region with an
    8.2 KiB contiguous run per SBUF partition (8 feature chunks x one
    513-row slab).
  - Device: for each 128-row block i, compute the near-diagonal Gram block
        G_i = X_blk^T @ X_blk'  in PSUM  ([128, 129], fp32 accum over 16
        feature chunks of 128).  diag(G_i)[p] = s_{128i+p} = ||x_l||^2,
        superdiag(G_i)[p] = c_{128i+p} = <x_l, x_{l+1}>.
    Two blocks share one PSUM bank ([128, 258]) so the masked
    multiply+reduce extraction on VectorE amortizes per-op overhead.
  - Host: combine s, c from all 8 cores in float64:
        diag_sim_l = c_l / (max(sqrt(s_l),eps) * max(sqrt(s_{l+1}),eps))
"""

import numpy as np
import ml_dtypes

B, L, D = 8, 4096, 2048
P = 128
W = P + 1                      # 129: Gram block width (incl. superdiag col)
NCHUNK = D // P                # 16 feature chunks
NBLK = L // P                  # 32 Gram blocks per core
EPS = 1e-12
IN_DT = "bfloat16"             # input dtype on device: bfloat16 | float8e4

NSLAB = 8                      # row slabs (DMA/pipeline granularity)
SLAB_ROWS = L // NSLAB         # 512
SLAB_COLS = SLAB_ROWS + 1      # 513 (one lookahead row; last is zero pad)
BLK_PER_SLAB = NBLK // NSLAB   # 4
CPD = 8                        # feature chunks packed per DMA
NGRP = NCHUNK // CPD           # 2 chunk groups

_cache = {}


def _build():
    import concourse.bass as bass
    import concourse.bacc as bacc
    import concourse.tile as tile
    from concourse import mybir

    nc = bacc.Bacc(
        "TRN2", target_bir_lowering=False, debug=False, num_devices=B
    )
    f32 = mybir.dt.float32
    in_dt = getattr(mybir.dt, IN_DT)

    xt_d = nc.dram_tensor(
        "xt", [NGRP, NSLAB, P, CPD * SLAB_COLS], in_dt, kind="ExternalInput"
    ).ap()
    mk_d = nc.dram_tensor("mk", [P, 2 * 2 * W], f32, kind="ExternalInput").ap()
    sc_d = nc.dram_tensor("sc", [P, 2 * NBLK], f32, kind="ExternalOutput").ap()

    with tile.TileContext(nc) as tc:
        with (
            tc.tile_pool(name="xin", bufs=1) as xin,
            tc.tile_pool(name="cst", bufs=1) as cst,
            tc.tile_pool(name="scr", bufs=4) as scr,
            tc.tile_pool(name="outp", bufs=1) as outp,
            tc.tile_pool(name="psum", bufs=8, space=bass.MemorySpace.PSUM) as psum,
        ):
            mk = cst.tile([P, 2 * 2 * W], f32, name="mk_sb")
            nc.sync.dma_start(out=mk, in_=mk_d)
            sc = outp.tile([P, 2 * NBLK], f32, name="sc_sb")

            # Input tiles, DMA'd slab-major so early row blocks are ready
            # while later slabs stream in.  One DMA = 8 feature chunks of
            # one slab = contiguous 1 MiB, 8.2 KiB per partition.
            xt = {}
            for j in range(NSLAB):
                for g in range(NGRP):
                    t = xin.tile(
                        [P, CPD * SLAB_COLS], in_dt,
                        tag=f"xt_{g}_{j}", name=f"xt_{g}_{j}",
                    )
                    nc.sync.dma_start(out=t, in_=xt_d[g, j])
                    xt[(g, j)] = t

            for tpair in range(NBLK // 2):
                pb = psum.tile([P, 2 * W], f32, tag="gram", name=f"gram_{tpair}")
                for u in range(2):
                    i = 2 * tpair + u
                    j, m = divmod(i, BLK_PER_SLAB)
                    for k in range(NCHUNK):
                        g, kk = divmod(k, CPD)
                        m0 = kk * SLAB_COLS + m * P
                        t = xt[(g, j)]
                        nc.tensor.matmul(
                            pb[:, u * W : (u + 1) * W],
                            t[:, m0 : m0 + P],        # lhsT: stationary
                            t[:, m0 : m0 + W],        # rhs: moving
                            start=(k == 0),
                            stop=(k == NCHUNK - 1),
                        )
                for h in range(2):  # 0 -> diag (s), 1 -> superdiag (c)
                    tmp = scr.tile(
                        [P, 2 * W], f32, tag="scr", name=f"scr_{tpair}_{h}"
                    )
                    col = h * NBLK + 2 * tpair
                    nc.vector.tensor_mul(
                        tmp, pb, mk[:, 2 * h * W : 2 * (h + 1) * W]
                    )
                    nc.vector.reduce_sum(
                        sc[:, col : col + 2],
                        tmp[:].rearrange("p (b c) -> p b c", b=2),
                        axis=mybir.AxisListType.X,
                    )

            nc.sync.dma_start(out=sc_d, in_=sc)
    nc.compile()
    return nc


def _make_masks():
    mk = np.zeros((P, 2, 2, W), np.float32)
    r = np.arange(P)
    mk[r, 0, :, r] = 1.0      # diag mask, replicated for both blocks
    mk[r, 1, :, r + 1] = 1.0  # superdiag mask
    return mk.reshape(P, 2 * 2 * W)


def _np_in_dt():
    return {"float8e4": ml_dtypes.float8_e4m3, "bfloat16": ml_dtypes.bfloat16}[IN_DT]


def _prep_inputs(x):
    """x: [B, L, D] float32 -> list of per-core input maps."""
    np_dt = _np_in_dt()
    mk = _make_masks()
    in_maps = []
    for b in range(B):
        xt = np.zeros((D, L + 1), dtype=np_dt)
        xt[:, :L] = np.ascontiguousarray(x[b].T).astype(np_dt)
        # arr[g, j, p, kk*SLAB_COLS + cc] = xt[128*(CPD*g+kk) + p,
        #                                      SLAB_ROWS*j + cc]
        arr = np.empty((NGRP, NSLAB, P, CPD * SLAB_COLS), dtype=np_dt)
        for j in range(NSLAB):
            sl = xt[:, SLAB_ROWS * j : SLAB_ROWS * j + SLAB_COLS]  # [D, 513]
            # [D, 513] -> [NGRP, CPD, P, 513] -> [NGRP, P, CPD, 513]
            r = sl.reshape(NGRP, CPD, P, SLAB_COLS).transpose(0, 2, 1, 3)
            arr[:, j] = r.reshape(NGRP, P, CPD * SLAB_COLS)
        in_maps.append({"xt": arr, "mk": mk})
    return in_maps


def _combine(results):
    total = 0.0
    for b in range(B):
        sc = np.asarray(results[b]["sc"], dtype=np.float64)  # [P, 2*NBLK]
        s = sc[:, :NBLK].T.reshape(-1)  # s_l at [l % P, l // P]
        c = sc[:, NBLK:].T.reshape(-1)
        n = np.maximum(np.sqrt(s), EPS)
        diag = c[: L - 1] / (n[: L - 1] * n[1:L])
        total += diag.sum()
    coherence = total / (B * (L - 1))
    return np.array(1.0 - coherence, dtype=np.float32)


def _run(x, trace=False):
    from concourse import bass_utils

    if "nc" not in _cache:
        _cache["nc"] = _build()
    nc = _cache["nc"]
    in_maps = _prep_inputs(np.asarray(x, dtype=np.float32))
    res = bass_utils.run_bass_kernel_spmd(
        nc, in_maps, core_ids=list(range(B)), trace=trace
    )
    return _combine(res.results), res


def kernel(hidden_states):
    out, _ = _run(hidden_states, trace=False)
    return out


# revision 10
# speedup vs baseline: 1.5370x; 1.0118x over previous
"""ActionCoherenceLoss kernel for 8 Trainium2 NeuronCores.

reference:
    norm = ||x||_2 along D; h = x / max(norm, eps)
    diag_sim[b, l] = <h[b,l], h[b,l+1]>          (l = 0..L-2)
    out = 1 - mean(diag_sim)                      (f32 scalar)

Strategy:
  - Data-parallel over batch: core b handles x[b] ([L=4096, D=2048]).
  - Host: transpose to x^T [D, L], pad one zero row -> [D, L+1], cast to
    bf16, and pack so each DMA is one contiguous ~1 MiB region with an
    8.2 KiB contiguous run per SBUF partition (8 feature chunks x one
    513-row slab).
  - Device: for each 128-row block i, compute the near-diagonal Gram block
        G_i = X_blk^T @ X_blk'  in PSUM  ([128, 129], fp32 accum over 16
        feature chunks of 128).  diag(G_i)[p] = s_{128i+p} = ||x_l||^2,
        superdiag(G_i)[p] = c_{128i+p} = <x_l, x_{l+1}>.
    Two blocks share one PSUM bank ([128, 258]) so the masked
    multiply+reduce extraction on VectorE amortizes per-op overhead.
  - Host: combine s, c from all 8 cores in float64:
        diag_sim_l = c_l / (max(sqrt(s_l),eps) * max(sqrt(s_{l+1}),eps))
"""

import numpy as np
import ml_dtypes

B, L, D = 8, 4096, 2048
P = 128
W = P + 1                      # 129: Gram block width (incl. superdiag col)
NCHUNK = D // P                # 16 feature chunks
NBLK = L // P                  # 32 Gram blocks per core
EPS = 1e-12
IN_DT = "float8e4"             # dtype in device DRAM: bfloat16 | float8e4
SB_DT = "bfloat16"             # dtype in SBUF (DMA casts if different)

NSLAB = 8                      # row slabs (DMA/pipeline granularity)
SLAB_ROWS = L // NSLAB         # 512
SLAB_COLS = SLAB_ROWS + 1      # 513 (one lookahead row; last is zero pad)
BLK_PER_SLAB = NBLK // NSLAB   # 4
CPD = 8                        # feature chunks packed per DMA
NGRP = NCHUNK // CPD           # 2 chunk groups

_cache = {}


def _build():
    import concourse.bass as bass
    import concourse.bacc as bacc
    import concourse.tile as tile
    from concourse import mybir

    nc = bacc.Bacc(
        "TRN2", target_bir_lowering=False, debug=False, num_devices=B
    )
    f32 = mybir.dt.float32
    in_dt = getattr(mybir.dt, IN_DT)
    sb_dt = getattr(mybir.dt, SB_DT)
    dma_in = nc.gpsimd.dma_start if IN_DT != SB_DT else nc.sync.dma_start

    xt_d = nc.dram_tensor(
        "xt", [NGRP, NSLAB, P, CPD * SLAB_COLS], in_dt, kind="ExternalInput"
    ).ap()
    mk_d = nc.dram_tensor("mk", [P, 2 * 2 * W], f32, kind="ExternalInput").ap()
    sc_d = nc.dram_tensor("sc", [P, 2 * NBLK], f32, kind="ExternalOutput").ap()

    with tile.TileContext(nc) as tc:
        with (
            tc.tile_pool(name="xin", bufs=1) as xin,
            tc.tile_pool(name="cst", bufs=1) as cst,
            tc.tile_pool(name="scr", bufs=4) as scr,
            tc.tile_pool(name="outp", bufs=1) as outp,
            tc.tile_pool(name="psum", bufs=8, space=bass.MemorySpace.PSUM) as psum,
        ):
            mk = cst.tile([P, 2 * 2 * W], f32, name="mk_sb")
            nc.sync.dma_start(out=mk, in_=mk_d)
            sc = outp.tile([P, 2 * NBLK], f32, name="sc_sb")

            # Input tiles, DMA'd slab-major so early row blocks are ready
            # while later slabs stream in.  One DMA = 8 feature chunks of
            # one slab = contiguous 1 MiB, 8.2 KiB per partition.
            xt = {}
            for j in range(NSLAB):
                for g in range(NGRP):
                    t = xin.tile(
                        [P, CPD * SLAB_COLS], sb_dt,
                        tag=f"xt_{g}_{j}", name=f"xt_{g}_{j}",
                    )
                    dma_in(out=t, in_=xt_d[g, j])
                    xt[(g, j)] = t

            for tpair in range(NBLK // 2):
                pb = psum.tile([P, 2 * W], f32, tag="gram", name=f"gram_{tpair}")
                for u in range(2):
                    i = 2 * tpair + u
                    j, m = divmod(i, BLK_PER_SLAB)
                    for k in range(NCHUNK):
                        g, kk = divmod(k, CPD)
                        m0 = kk * SLAB_COLS + m * P
                        t = xt[(g, j)]
                        nc.tensor.matmul(
                            pb[:, u * W : (u + 1) * W],
                            t[:, m0 : m0 + P],        # lhsT: stationary
                            t[:, m0 : m0 + W],        # rhs: moving
                            start=(k == 0),
                            stop=(k == NCHUNK - 1),
                        )
                for h in range(2):  # 0 -> diag (s), 1 -> superdiag (c)
                    tmp = scr.tile(
                        [P, 2 * W], f32, tag="scr", name=f"scr_{tpair}_{h}"
                    )
                    col = h * NBLK + 2 * tpair
                    nc.vector.tensor_mul(
                        tmp, pb, mk[:, 2 * h * W : 2 * (h + 1) * W]
                    )
                    nc.vector.reduce_sum(
                        sc[:, col : col + 2],
                        tmp[:].rearrange("p (b c) -> p b c", b=2),
                        axis=mybir.AxisListType.X,
                    )

            nc.sync.dma_start(out=sc_d, in_=sc)
    nc.compile()
    return nc


def _make_masks():
    mk = np.zeros((P, 2, 2, W), np.float32)
    r = np.arange(P)
    mk[r, 0, :, r] = 1.0      # diag mask, replicated for both blocks
    mk[r, 1, :, r + 1] = 1.0  # superdiag mask
    return mk.reshape(P, 2 * 2 * W)


def _np_in_dt():
    return {"float8e4": ml_dtypes.float8_e4m3, "bfloat16": ml_dtypes.bfloat16}[IN_DT]


def _prep_inputs(x):
    """x: [B, L, D] float32 -> list of per-core input maps."""
    np_dt = _np_in_dt()
    mk = _make_masks()
    in_maps = []
    for b in range(B):
        xt = np.zeros((D, L + 1), dtype=np_dt)
        xt[:, :L] = np.ascontiguousarray(x[b].T).astype(np_dt)
        # arr[g, j, p, kk*SLAB_COLS + cc] = xt[128*(CPD*g+kk) + p,
        #                                      SLAB_ROWS*j + cc]
        arr = np.empty((NGRP, NSLAB, P, CPD * SLAB_COLS), dtype=np_dt)
        for j in range(NSLAB):
            sl = xt[:, SLAB_ROWS * j : SLAB_ROWS * j + SLAB_COLS]  # [D, 513]
            # [D, 513] -> [NGRP, CPD, P, 513] -> [NGRP, P, CPD, 513]
            r = sl.reshape(NGRP, CPD, P, SLAB_COLS).transpose(0, 2, 1, 3)
            arr[:, j] = r.reshape(NGRP, P, CPD * SLAB_COLS)
        in_maps.append({"xt": arr, "mk": mk})
    return in_maps


def _combine(results):
    total = 0.0
    for b in range(B):
        sc = np.asarray(results[b]["sc"], dtype=np.float64)  # [P, 2*NBLK]
        s = sc[:, :NBLK].T.reshape(-1)  # s_l at [l % P, l // P]
        c = sc[:, NBLK:].T.reshape(-1)
        n = np.maximum(np.sqrt(s), EPS)
        diag = c[: L - 1] / (n[: L - 1] * n[1:L])
        total += diag.sum()
    coherence = total / (B * (L - 1))
    return np.array(1.0 - coherence, dtype=np.float32)


def _run(x, trace=False):
    from concourse import bass_utils

    if "nc" not in _cache:
        _cache["nc"] = _build()
    nc = _cache["nc"]
    in_maps = _prep_inputs(np.asarray(x, dtype=np.float32))
    res = bass_utils.run_bass_kernel_spmd(
        nc, in_maps, core_ids=list(range(B)), trace=trace
    )
    return _combine(res.results), res


def kernel(hidden_states):
    out, _ = _run(hidden_states, trace=False)
    return out


# revision 11
# speedup vs baseline: 2.3304x; 1.5162x over previous
"""ActionCoherenceLoss kernel for 8 Trainium2 NeuronCores.

reference:
    norm = ||x||_2 along D; h = x / max(norm, eps)
    diag_sim[b, l] = <h[b,l], h[b,l+1]>          (l = 0..L-2)
    out = 1 - mean(diag_sim)                      (f32 scalar)

Strategy:
  - Data-parallel over batch: core b handles x[b] ([L=4096, D=2048]).
  - Host: transpose to x^T [D, L], pad one zero row -> [D, L+1], cast to
    fp8 e4m3 (final scalar rel-err ~3e-6), pack one 513-row slab of all 16
    feature chunks per DMA (contiguous ~1 MiB, 8.4 KiB per partition;
    row-stride padded to 528 so fp8 DoubleRow APs are 16B-aligned).
  - Device: for each 128-row block i, compute the near-diagonal Gram block
        G_i = X_blk^T @ X_blk'  in PSUM  ([128, 129], fp32 accum) with 8
    fp8 DoubleRow matmuls (256-deep contraction each).
        diag(G_i)[p] = s_{128i+p} = ||x_l||^2,
        superdiag(G_i)[p] = c_{128i+p} = <x_l, x_{l+1}>.
    Two blocks share one PSUM bank ([128, 258]) so the masked
    multiply+reduce extraction on VectorE amortizes per-op overhead.
  - Host: combine s, c from all 8 cores in float64:
        diag_sim_l = c_l / (max(sqrt(s_l),eps) * max(sqrt(s_{l+1}),eps))
"""

import numpy as np
import ml_dtypes

B, L, D = 8, 4096, 2048
P = 128
W = P + 1                      # 129: Gram block width (incl. superdiag col)
NCHUNK = D // P                # 16 feature chunks
NBLK = L // P                  # 32 Gram blocks per core
EPS = 1e-12
IN_DT = "float8e4"             # dtype in DRAM + SBUF: bfloat16 | float8e4
DOUBLE_ROW = True              # fp8 DoubleRow matmuls (half the MM count)

NSLAB = 8                      # row slabs (DMA/pipeline granularity)
SLAB_ROWS = L // NSLAB         # 512
SLAB_COLS = SLAB_ROWS + 1      # 513 valid columns (last is zero pad row)
COLSTRIDE = 528                # stored stride: %16==0 for DoubleRow APs
BLK_PER_SLAB = NBLK // NSLAB   # 4

_cache = {}


def _build():
    import concourse.bass as bass
    import concourse.bacc as bacc
    import concourse.tile as tile
    from concourse import mybir

    nc = bacc.Bacc(
        "TRN2", target_bir_lowering=False, debug=False, num_devices=B
    )
    f32 = mybir.dt.float32
    in_dt = getattr(mybir.dt, IN_DT)

    xt_d = nc.dram_tensor(
        "xt", [NSLAB, P, NCHUNK * COLSTRIDE], in_dt, kind="ExternalInput"
    ).ap()
    mk_d = nc.dram_tensor("mk", [P, 2 * 2 * W], f32, kind="ExternalInput").ap()
    sc_d = nc.dram_tensor("sc", [P, 2 * NBLK], f32, kind="ExternalOutput").ap()

    with tile.TileContext(nc) as tc:
        with (
            tc.tile_pool(name="xin", bufs=1) as xin,
            tc.tile_pool(name="cst", bufs=1) as cst,
            tc.tile_pool(name="scr", bufs=4) as scr,
            tc.tile_pool(name="outp", bufs=1) as outp,
            tc.tile_pool(name="psum", bufs=8, space=bass.MemorySpace.PSUM) as psum,
        ):
            mk = cst.tile([P, 2 * 2 * W], f32, name="mk_sb")
            nc.sync.dma_start(out=mk, in_=mk_d)
            sc = outp.tile([P, 2 * NBLK], f32, name="sc_sb")

            # One DMA per slab: all 16 feature chunks, contiguous ~1 MiB.
            xt = []
            for j in range(NSLAB):
                t = xin.tile(
                    [P, NCHUNK * COLSTRIDE], in_dt,
                    tag=f"xt_{j}", name=f"xt_{j}",
                )
                nc.sync.dma_start(out=t, in_=xt_d[j])
                xt.append(t)

            for tpair in range(NBLK // 2):
                pb = psum.tile([P, 2 * W], f32, tag="gram", name=f"gram_{tpair}")
                for u in range(2):
                    i = 2 * tpair + u
                    j, m = divmod(i, BLK_PER_SLAB)
                    m0 = m * P
                    t = xt[j]
                    out_ap = pb[:, u * W : (u + 1) * W]
                    if DOUBLE_ROW:
                        t3 = t[:].rearrange("p (c w) -> p c w", w=COLSTRIDE)
                        for k in range(NCHUNK // 2):
                            lhsT = t3[:, 2 * k : 2 * k + 2, m0 : m0 + P]
                            rhs = t3[:, 2 * k : 2 * k + 2, m0 : m0 + W]
                            nc.tensor.matmul(
                                out_ap, lhsT, rhs,
                                start=(k == 0),
                                stop=(k == NCHUNK // 2 - 1),
                                perf_mode=mybir.MatmulPerfMode.DoubleRow,
                            )
                    else:
                        for k in range(NCHUNK):
                            c0 = k * COLSTRIDE + m0
                            nc.tensor.matmul(
                                out_ap,
                                t[:, c0 : c0 + P],
                                t[:, c0 : c0 + W],
                                start=(k == 0),
                                stop=(k == NCHUNK - 1),
                            )
                for h in range(2):  # 0 -> diag (s), 1 -> superdiag (c)
                    tmp = scr.tile(
                        [P, 2 * W], f32, tag="scr", name=f"scr_{tpair}_{h}"
                    )
                    col = h * NBLK + 2 * tpair
                    nc.vector.tensor_mul(
                        tmp, pb, mk[:, 2 * h * W : 2 * (h + 1) * W]
                    )
                    nc.vector.reduce_sum(
                        sc[:, col : col + 2],
                        tmp[:].rearrange("p (b c) -> p b c", b=2),
                        axis=mybir.AxisListType.X,
                    )

            nc.sync.dma_start(out=sc_d, in_=sc)
    nc.compile()
    return nc


def _make_masks():
    mk = np.zeros((P, 2, 2, W), np.float32)
    r = np.arange(P)
    mk[r, 0, :, r] = 1.0      # diag mask, replicated for both blocks
    mk[r, 1, :, r + 1] = 1.0  # superdiag mask
    return mk.reshape(P, 2 * 2 * W)


def _np_in_dt():
    return {"float8e4": ml_dtypes.float8_e4m3, "bfloat16": ml_dtypes.bfloat16}[IN_DT]


def _prep_inputs(x):
    """x: [B, L, D] float32 -> list of per-core input maps."""
    np_dt = _np_in_dt()
    mk = _make_masks()
    in_maps = []
    for b in range(B):
        xt = np.zeros((D, L + 1), dtype=np_dt)
        xt[:, :L] = np.ascontiguousarray(x[b].T).astype(np_dt)
        # arr[j, p, k*COLSTRIDE + cc] = xt[128k + p, SLAB_ROWS*j + cc]
        arr = np.zeros((NSLAB, P, NCHUNK * COLSTRIDE), dtype=np_dt)
        a4 = arr.reshape(NSLAB, P, NCHUNK, COLSTRIDE)
        for j in range(NSLAB):
            sl = xt[:, SLAB_ROWS * j : SLAB_ROWS * j + SLAB_COLS]  # [D, 513]
            a4[j, :, :, :SLAB_COLS] = sl.reshape(NCHUNK, P, SLAB_COLS).transpose(
                1, 0, 2
            )
        in_maps.append({"xt": arr, "mk": mk})
    return in_maps


def _combine(results):
    total = 0.0
    for b in range(B):
        sc = np.asarray(results[b]["sc"], dtype=np.float64)  # [P, 2*NBLK]
        s = sc[:, :NBLK].T.reshape(-1)  # s_l at [l % P, l // P]
        c = sc[:, NBLK:].T.reshape(-1)
        n = np.maximum(np.sqrt(s), EPS)
        diag = c[: L - 1] / (n[: L - 1] * n[1:L])
        total += diag.sum()
    coherence = total / (B * (L - 1))
    return np.array(1.0 - coherence, dtype=np.float32)


def _run(x, trace=False):
    from concourse import bass_utils

    if "nc" not in _cache:
        _cache["nc"] = _build()
    nc = _cache["nc"]
    in_maps = _prep_inputs(np.asarray(x, dtype=np.float32))
    res = bass_utils.run_bass_kernel_spmd(
        nc, in_maps, core_ids=list(range(B)), trace=trace
    )
    return _combine(res.results), res


def kernel(hidden_states):
    out, _ = _run(hidden_states, trace=False)
    return out
